# revision 3
# baseline (speedup 1.0000x reference)
"""Trainium2 Bass kernel for y = x*x - 1 (elementwise, f32).

Full input x: (8192, 16384) f32. Sharded row-wise across 8 NeuronCores
(data parallel, no communication): each core processes a (1024, 16384)
slice. Memory-bound: per core 64 MiB in + 64 MiB out at the ~358 GB/s
per-core HBM limit => ~375 us floor; measured ~400 us/pass steady state.

Per-core pipeline (Tile-scheduled): 8 row-block tiles of [128, 16384]
(8 MiB, fully contiguous in DRAM => maximally efficient DMA descriptors),
double-buffered: HWDGE DMA load -> ScalarE Square (in-place) -> VectorE
tensor_scalar add -1 (in-place, 2x mode for f32 SBUF) -> HWDGE DMA store.
Both compute engines run far under the DMA roofline, so DMA stays the
bottleneck.

Swept alternatives (K-pass For_i loop, wall-clock slope): tile free dim
{2048..16384} x bufs {2..10}, store on gpsimd/scalar ring, DVE-only
compute, chunked stores - all within noise (~395-435 us); this config
measured best and most consistent.

Second sweep session (17 variants) confirmed this config is the optimum
and explained why:
- load-only diagnostic: 8x8MiB reads take 199us = 187.5us line rate
  (358 GB/s) + ~2us completion-receipt latency per buffer reuse. So the
  HW can hit the roofline; the ~20us/pass gap is receipt gaps + R/W
  turnaround, mostly hidden here by the compute pipeline.
- load+store with NO compute: 419us (worse than 395!) - compute latency
  is what hides the store->load receipt gaps on the serial ring.
- bufs=3..6: +22..26us regardless of other knobs. bufs=2 wins.
- stores on the 2nd HWDGE ring (ACT): +16..46us. ACT-issued DMA triggers
  serialize with Square activations, and concurrent R/W streams on two
  rings interleave at packet granularity (max HBM turnaround) instead of
  this config's clean alternating 8 MiB read/write bursts on one ring.
- both ops on DVE (tensor_tensor + tensor_scalar): ~25.5us/tile > 23.4us
  DMA rate -> compute-bound, +29us. Square on gpsimd: +40..120us (SWDGE
  descriptor-ring contention with DVE 2-port mode).
- [128, 131072] flat view (512KiB/partition DRAM stride, 12MiB DMAs):
  259 GB/s - the natural 64KiB-stride layout matters to HBM locality.
- edge-tapered tile schedules (small first/last chunks): +19us - the
  pass edges were already free (ring stays saturated); extra DMAs just
  add per-DMA overhead.
- closing diagnostics: pure-store pass 194.3us, pure-load pass 199.4us,
  and 194.3 + 199.4 = 393.7 ~= this kernel's 394.6us mixed pass. The
  single serial ring composes the R and W streams with ZERO mixing
  penalty; each stream is at line rate + ~0.7-0.9us/DMA HWDGE
  first-byte bubble. Store-halving (+8 DMAs) costs +19us and a lone
  extra ramp DMA costs +31us, confirming per-DMA bubbles (not
  compute->store coupling) as the only residual cost. loads-only with
  bufs=3 shows NO slowdown, so the bufs>=3 curse is SBUF port pressure
  from 3 concurrent clients (DMA-in, compute, DMA-out) per partition -
  bufs=2 keeps it to 2 by construction.
"""

import sys

import numpy as np

if "/opt/trn_rl_repo" not in sys.path:
    sys.path.insert(0, "/opt/trn_rl_repo")

M, N = 8192, 16384
N_CORES = 8
ROWS_PER_CORE = M // N_CORES  # 1024
P = 128  # SBUF partitions
FREE = 16384  # tile free-dim elements (8 MiB f32 tiles, contiguous rows)
BUFS = 2

_nc_cache = {}


def _build():
    key = (ROWS_PER_CORE, N, FREE, BUFS)
    if key in _nc_cache:
        return _nc_cache[key]

    import concourse.mybir as mybir
    from concourse import bacc
    from concourse.tile import TileContext

    # Bacc (not plain Bass): its finalize() runs generate_event_semaphores,
    # which splits multi-semaphore waits into standalone event instructions.
    # Raw Bass modules with >1 wait on a DMA fail walrus codegen ("Too many
    # sync wait commands").
    nc = bacc.Bacc("TRN2")
    x = nc.dram_tensor(
        "x", [ROWS_PER_CORE, N], mybir.dt.float32, kind="ExternalInput"
    )
    y = nc.dram_tensor(
        "y", [ROWS_PER_CORE, N], mybir.dt.float32, kind="ExternalOutput"
    )
    xv = x.rearrange("(n p) m -> n p m", p=P)  # [8, 128, 16384]
    yv = y.rearrange("(n p) m -> n p m", p=P)
    n_blocks = ROWS_PER_CORE // P
    n_f = N // FREE

    with TileContext(nc) as tc:
        with tc.tile_pool(name="buf", bufs=BUFS) as pool:
            for nb in range(n_blocks):
                for f in range(n_f):
                    t = pool.tile([P, FREE], mybir.dt.float32)
                    src = xv[nb, :, f * FREE : (f + 1) * FREE]
                    dst = yv[nb, :, f * FREE : (f + 1) * FREE]
                    nc.sync.dma_start(t[:], src)
                    nc.scalar.activation(
                        t[:], t[:], mybir.ActivationFunctionType.Square
                    )
                    nc.vector.tensor_scalar_add(t[:], t[:], -1.0)
                    nc.sync.dma_start(dst, t[:])

    if not nc.is_finalized():
        nc.finalize()
    _nc_cache[key] = nc
    return nc


def kernel(x):
    from concourse.bass_utils import run_bass_kernel_spmd

    x = np.ascontiguousarray(np.asarray(x, dtype=np.float32))
    assert x.shape == (M, N), x.shape

    nc = _build()
    shards = np.split(x, N_CORES, axis=0)
    in_maps = [{"x": s} for s in shards]
    res = run_bass_kernel_spmd(nc, in_maps, core_ids=list(range(N_CORES)))
    out = np.concatenate([r["y"] for r in res.results], axis=0)
    return out.astype(np.float32, copy=False)

